# revision 14
# baseline (speedup 1.0000x reference)
"""AttnOutputDecoder Trainium2 kernel.

Sharding: data-parallel over batch B=16 across 8 cores (2 batches/core).
Each core: LSTM (transposed, W-stationary bf16 matmuls) -> Bahdanau
attention (tanh via ACT per-partition bias) -> output proj -> full-vocab
projection (bf16, streamed). Host does embedding gather, transposes,
bf16 casts, and the input projection x @ W_ih.T (not recurrent).
"""

import numpy as np
import ml_dtypes

import concourse.bass as bass
import concourse.mybir as mybir
import concourse.tile as tile
from concourse import bacc
from concourse import bass_utils

BF16 = ml_dtypes.bfloat16
F32 = mybir.dt.float32
BF = mybir.dt.bfloat16
AF = mybir.ActivationFunctionType
ALU = mybir.AluOpType

B, T, S, D, V = 16, 64, 128, 512, 32000
NC = 8
BL = B // NC          # local batches per core = 2
R = BL * T            # local rows = 128
G4 = 4 * D            # 2048 gates
KC = D // 128         # 4 contraction chunks
VBLK = 512

_cached = {}


def _build_nc():
    nc = bacc.Bacc("TRN2", target_bir_lowering=False, debug=False,
                   num_devices=NC)

    def din(name, shape, dt):
        return nc.dram_tensor(name, shape, dt, kind="ExternalInput").ap()

    t_xg = din("xg", [128, 16 * 128], F32)          # [p,(j,t,b)] gate-chunk j
    t_whh = din("whh", [128, KC * G4], BF)           # [p,(kc,g)] = W_hh.T re
    t_h0 = din("h0", [128, KC * BL], F32)            # [p,(kc,b)]
    t_c0 = din("c0", [128, KC * BL], F32)
    t_encT = din("encT", [128, KC * BL * S], BF)     # [p,(kc,b,s)]
    t_enc = din("enc", [128, BL * D], BF)            # [s,(b,d)]
    t_whT = din("whT", [128, KC * D], BF)            # [p,(kc,d)] Wh_w.T re
    t_wsT = din("wsT", [128, KC * D], BF)
    t_vw1 = din("vw1", [128, KC * D], BF)            # (V_w[:,:D]).T re
    t_vw2 = din("vw2", [128, KC * D], BF)
    t_wsb = din("wsb", [128, KC], F32)               # Ws_b chunks
    t_vb = din("vb", [128, KC], F32)                 # V_b chunks
    t_vt = din("vt", [128, KC], BF)                  # vt_w chunks
    t_vpt = din("vpt", [128, KC * V], BF)            # [p,(kc,v)] Vp_w.T re
    t_vpb = din("vpb", [1, V], BF)
    t_ones = din("ones", [1, 128], BF)
    t_ident = din("ident", [128, 128], BF)
    t_out = nc.dram_tensor("out", [R, V], F32, kind="ExternalOutput").ap()

    with tile.TileContext(nc) as tc:
        with (
            tc.tile_pool(name="const", bufs=1) as cp,
            tc.tile_pool(name="state", bufs=1) as sp,
            tc.tile_pool(name="gates", bufs=2) as gp,
            tc.tile_pool(name="attn", bufs=3) as ap_,
            tc.tile_pool(name="voc", bufs=3) as vp,
            tc.tile_pool(name="ps_g", bufs=2, space="PSUM") as ppg,
            tc.tile_pool(name="ps_e", bufs=1, space="PSUM") as ppe,
            tc.tile_pool(name="ps_sm", bufs=2, space="PSUM") as pps,
            tc.tile_pool(name="ps_v", bufs=2, space="PSUM") as ppv,
        ):
            # ---- resident constants ----
            whh = cp.tile([128, KC * G4], BF)
            nc.sync.dma_start(out=whh[:], in_=t_whh[:])
            xg = cp.tile([128, 16 * 128], F32)
            nc.sync.dma_start(out=xg[:], in_=t_xg[:])
            encT = cp.tile([128, KC * BL * S], BF)
            nc.sync.dma_start(out=encT[:], in_=t_encT[:])
            enc = cp.tile([128, BL * D], BF)
            nc.sync.dma_start(out=enc[:], in_=t_enc[:])
            whT = cp.tile([128, KC * D], BF)
            nc.sync.dma_start(out=whT[:], in_=t_whT[:])
            wsT = cp.tile([128, KC * D], BF)
            nc.sync.dma_start(out=wsT[:], in_=t_wsT[:])
            vw1 = cp.tile([128, KC * D], BF)
            nc.sync.dma_start(out=vw1[:], in_=t_vw1[:])
            vw2 = cp.tile([128, KC * D], BF)
            nc.sync.dma_start(out=vw2[:], in_=t_vw2[:])
            wsb = cp.tile([128, KC], F32)
            nc.sync.dma_start(out=wsb[:], in_=t_wsb[:])
            vb = cp.tile([128, KC], F32)
            nc.sync.dma_start(out=vb[:], in_=t_vb[:])
            vt = cp.tile([128, KC], BF)
            nc.sync.dma_start(out=vt[:], in_=t_vt[:])
            ones = cp.tile([1, 128], BF)
            nc.sync.dma_start(out=ones[:], in_=t_ones[:])
            ident = cp.tile([128, 128], BF)
            nc.sync.dma_start(out=ident[:], in_=t_ident[:])

            # ---- state ----
            h = sp.tile([128, KC * BL], F32)    # h_T [p,(kc,b)]
            c = sp.tile([128, KC * BL], F32)
            nc.sync.dma_start(out=h[:], in_=t_h0[:])
            nc.sync.dma_start(out=c[:], in_=t_c0[:])
            hbf = sp.tile([128, KC * BL], BF)
            nc.vector.tensor_copy(out=hbf[:], in_=h[:])
            outT = sp.tile([128, KC * BL * T], BF)   # [p,(kc,b,t)] all h's

            xg4 = xg[:].rearrange("p (j t b) -> p j t b", j=16, t=T, b=BL)
            outT4 = outT[:].rearrange("p (kc b t) -> p kc b t", kc=KC, b=BL,
                                      t=T)

            # ---- vocab weight prefetch (hidden under compute) ----
            NPRE = 32
            vpt4 = t_vpt[:].rearrange("p (kc v) -> p kc v", kc=KC, v=V)
            vpre = cp.tile([128, NPRE * KC * VBLK], BF)
            vpre4 = vpre[:].rearrange("p (i kc v) -> p i kc v", i=NPRE,
                                      kc=KC, v=VBLK)
            for i in range(NPRE):
                for kc in range(KC):
                    nc.sync.dma_start(out=vpre4[:, i, kc, :],
                                      in_=vpt4[:, kc, i * VBLK:(i + 1) * VBLK])

            # ====== wh = enc @ Wh_w.T  (before LSTM; -> sbuf bf16) ======
            whs = sp.tile([128, BL * KC * 128], BF)   # [p,(b,dc,s)]
            for b in range(BL):
                whp = ppv.tile([128, VBLK], F32, tag="lps")
                for dc in range(KC):
                    for kc in range(KC):
                        nc.tensor.matmul(
                            out=whp[:, dc * 128:(dc + 1) * 128],
                            lhsT=whT[:, kc * D + dc * 128: kc * D + (dc + 1) * 128],
                            rhs=encT[:, (kc * BL + b) * S:(kc * BL + b + 1) * S],
                            start=(kc == 0), stop=(kc == KC - 1))
                nc.vector.tensor_copy(out=whs[:, b * 512:(b + 1) * 512],
                                      in_=whp[:])

            wst = sp.tile([128, KC * BL * T], F32)   # [p,(dc,b,t)]
            eps0 = ppe.tile([S, T], F32, tag="e0")
            eps1 = ppe.tile([S, T], F32, tag="e1")
            epss = [eps0, eps1]

            # ========== LSTM + blocked attention-score overlap ==========
            def emit_score(b, t):
                for dc in range(KC):
                    th = ap_.tile([128, S], BF, tag="th", name=f"th{b}_{t}_{dc}")
                    nc.scalar.activation(
                        out=th[:],
                        in_=whs[:, b * 512 + dc * 128:
                                b * 512 + (dc + 1) * 128],
                        func=AF.Tanh,
                        bias=wst[:, (dc * BL + b) * T + t:
                                 (dc * BL + b) * T + t + 1])
                    nc.tensor.matmul(out=epss[b][:, t:t + 1],
                                     lhsT=th[:], rhs=vt[:, dc:dc + 1],
                                     start=(dc == 0), stop=(dc == KC - 1))

            pending = []
            TB = 16
            for blk in range(T // TB):
                tlo = blk * TB
                for t in range(tlo, tlo + TB):
                    gps = ppg.tile([128, 16 * BL], F32, tag="gps")
                    for j in range(16):
                        for kc in range(KC):
                            nc.tensor.matmul(
                                out=gps[:, j * BL:(j + 1) * BL],
                                lhsT=whh[:, kc * G4 + j * 128:
                                          kc * G4 + (j + 1) * 128],
                                rhs=hbf[:, kc * BL:(kc + 1) * BL],
                                start=(kc == 0), stop=(kc == KC - 1))
                    gs = gp.tile([128, 16 * BL], F32, tag="gs")
                    gps3 = gps[:].rearrange("p (j b) -> p j b", j=16, b=BL)
                    gs3 = gs[:].rearrange("p (j b) -> p j b", j=16, b=BL)
                    nc.vector.tensor_add(out=gs3, in0=gps3, in1=xg4[:, :, t, :])
                    sio = gp.tile([128, 16 * BL], F32, tag="sio")
                    nc.scalar.activation(out=sio[:, 0:8 * BL],
                                         in_=gs[:, 0:8 * BL], func=AF.Sigmoid)
                    nc.scalar.activation(out=sio[:, 12 * BL:16 * BL],
                                         in_=gs[:, 12 * BL:16 * BL],
                                         func=AF.Sigmoid)
                    nc.scalar.activation(out=sio[:, 8 * BL:12 * BL],
                                         in_=gs[:, 8 * BL:12 * BL],
                                         func=AF.Tanh)
                    t1 = gp.tile([128, KC * BL], F32, tag="t1")
                    t2 = gp.tile([128, KC * BL], F32, tag="t2")
                    nc.vector.tensor_mul(out=t1[:], in0=sio[:, 4 * BL:8 * BL],
                                         in1=c[:])
                    nc.vector.tensor_mul(out=t2[:], in0=sio[:, 0:4 * BL],
                                         in1=sio[:, 8 * BL:12 * BL])
                    nc.vector.tensor_add(out=c[:], in0=t1[:], in1=t2[:])
                    tc_ = gp.tile([128, KC * BL], F32, tag="tc")
                    nc.scalar.activation(out=tc_[:], in_=c[:], func=AF.Tanh)
                    nc.vector.tensor_mul(out=h[:],
                                         in0=sio[:, 12 * BL:16 * BL],
                                         in1=tc_[:])
                    nc.vector.tensor_copy(out=hbf[:], in_=h[:])
                    hbf3 = hbf[:].rearrange("p (kc b) -> p kc b", kc=KC, b=BL)
                    nc.vector.tensor_copy(out=outT4[:, :, :, t], in_=hbf3)
                    for _ in range(min(8, len(pending))):
                        emit_score(*pending.pop(0))

                # ws for this t-block
                for b in range(BL):
                    for dc in range(KC):
                        wps = pps.tile([128, TB], F32, tag="sm")
                        for kc in range(KC):
                            nc.tensor.matmul(
                                out=wps[:],
                                lhsT=wsT[:, kc * D + dc * 128:
                                         kc * D + (dc + 1) * 128],
                                rhs=outT[:, (kc * BL + b) * T + tlo:
                                         (kc * BL + b) * T + tlo + TB],
                                start=(kc == 0), stop=(kc == KC - 1))
                        nc.vector.tensor_scalar(
                            out=wst[:, (dc * BL + b) * T + tlo:
                                    (dc * BL + b) * T + tlo + TB],
                            in0=wps[:], scalar1=wsb[:, dc:dc + 1],
                            scalar2=None, op0=ALU.add)

                # queue this block's score tasks; emitted interleaved
                # with the next block's LSTM steps (keeps ACT round-robin)
                pending.extend((b, t) for b in range(BL)
                               for t in range(tlo, tlo + TB))

            # ============ scores, softmax, context, out2 ============
            ctxT = sp.tile([128, BL * KC * T], BF)   # [p,(b,dc,t)]
            o2T = sp.tile([128, KC * BL * T], BF)    # [p,(ec,b,t)]
            while pending:
                emit_score(*pending.pop(0))

            for b in range(BL):
                eps = epss[b]
                # softmax over s; |e| is small so no max-subtract needed
                ebf = ap_.tile([S, T], BF, tag="ebf")
                nc.scalar.activation(out=ebf[:], in_=eps[:], func=AF.Exp)
                # transpose exp(e).T -> [t, s]
                etp = pps.tile([T, S], BF, tag="sm")
                nc.tensor.transpose(out=etp[:], in_=ebf[:],
                                    identity=ident[:, :])
                ssum = ap_.tile([T, 1], F32, tag="ssum")
                nc.vector.tensor_reduce(out=ssum[:], in_=etp[:],
                                        axis=mybir.AxisListType.X, op=ALU.add)
                rsum = ap_.tile([T, 1], F32, tag="rsum")
                nc.vector.reciprocal(out=rsum[:], in_=ssum[:])
                abf = ap_.tile([T, S], BF, tag="abf")
                nc.vector.tensor_scalar_mul(out=abf[:], in0=etp[:],
                                            scalar1=rsum[:])
                # transpose a -> [s, t]
                atp = pps.tile([S, T], BF, tag="sm")
                nc.tensor.transpose(out=atp[:], in_=abf[:],
                                    identity=ident[0:T, 0:T])
                atb = ap_.tile([S, T], BF, tag="atb")
                nc.vector.tensor_copy(out=atb[:], in_=atp[:])
                # context: ctxT[d,t] = enc.T @ a
                for dc in range(KC):
                    cps = pps.tile([128, T], F32, tag="sm")
                    nc.tensor.matmul(out=cps[:],
                                     lhsT=enc[:, b * D + dc * 128:
                                              b * D + (dc + 1) * 128],
                                     rhs=atb[:], start=True, stop=True)
                    nc.vector.tensor_copy(
                        out=ctxT[:, (b * KC + dc) * T:(b * KC + dc + 1) * T],
                        in_=cps[:])
                # out2 = [ctx|out] @ V_w.T + V_b   (transposed)
                for ec in range(KC):
                    ops = pps.tile([128, T], F32, tag="sm")
                    for kc in range(KC):
                        nc.tensor.matmul(
                            out=ops[:],
                            lhsT=vw1[:, kc * D + ec * 128: kc * D + (ec + 1) * 128],
                            rhs=ctxT[:, (b * KC + kc) * T:(b * KC + kc + 1) * T],
                            start=(kc == 0), stop=False)
                    for kc in range(KC):
                        nc.tensor.matmul(
                            out=ops[:],
                            lhsT=vw2[:, kc * D + ec * 128: kc * D + (ec + 1) * 128],
                            rhs=outT[:, (kc * BL + b) * T:(kc * BL + b + 1) * T],
                            start=False, stop=(kc == KC - 1))
                    nc.vector.tensor_scalar(
                        out=o2T[:, (ec * BL + b) * T:(ec * BL + b + 1) * T],
                        in0=ops[:], scalar1=vb[:, ec:ec + 1], scalar2=None,
                        op0=ALU.add)

            # ================= vocab projection =================
            for ib, v0 in enumerate(range(0, V, VBLK)):
                w = min(VBLK, V - v0)
                if ib < NPRE:
                    vsrc = vpre4[:, ib]
                else:
                    vps = vp.tile([128, KC, VBLK], BF, tag="vps")
                    for kc in range(KC):
                        nc.sync.dma_start(out=vps[:, kc, :w],
                                          in_=vpt4[:, kc, v0:v0 + w])
                    vsrc = vps
                vpbt = vp.tile([1, VBLK], BF, tag="vpbt")
                nc.sync.dma_start(out=vpbt[:, :w], in_=t_vpb[:, v0:v0 + w])
                lps = ppv.tile([128, VBLK], F32, tag="lps")
                for kc in range(KC):
                    nc.tensor.matmul(out=lps[:, :w],
                                     lhsT=o2T[:, kc * 128:(kc + 1) * 128],
                                     rhs=vsrc[:, kc, :w],
                                     start=(kc == 0), stop=False)
                nc.tensor.matmul(out=lps[:, :w], lhsT=ones[:],
                                 rhs=vpbt[:, :w], start=False, stop=True)
                lsb = vp.tile([128, VBLK], F32, tag="lsb")
                if ib % 2 == 0:
                    nc.scalar.copy(out=lsb[:, :w], in_=lps[:, :w])
                else:
                    nc.vector.tensor_copy(out=lsb[:, :w], in_=lps[:, :w])
                nc.sync.dma_start(out=t_out[:, v0:v0 + w], in_=lsb[:, :w])

    nc.compile()
    return nc


def _prep_in_maps(inputs):
    inp = {k: np.asarray(v) for k, v in inputs.items()}
    words = inp["words"].astype(np.int64)
    enc = inp["encoder_output"].astype(np.float32)
    pre_h, cell = inp["pre_h"], inp["cell"]
    emb = inp["emb"]
    W_ih, W_hh = inp["W_ih"], inp["W_hh"]
    b_ih, b_hh = inp["b_ih"], inp["b_hh"]
    Wh_w = inp["Wh_w"]
    Ws_w, Ws_b = inp["Ws_w"], inp["Ws_b"]
    vt_w = inp["vt_w"]
    V_w, V_b = inp["V_w"], inp["V_b"]
    Vp_w, Vp_b = inp["Vp_w"], inp["Vp_b"]

    def re_lhsT(m):  # [512, N] -> [128, 4*N] chunk-major, bf16
        n = m.shape[1]
        return np.ascontiguousarray(
            m.reshape(4, 128, n).transpose(1, 0, 2).reshape(128, 4 * n)
        ).astype(BF16)

    whh_re = re_lhsT(np.ascontiguousarray(W_hh.T))
    whT_re = re_lhsT(np.ascontiguousarray(Wh_w.T))
    wsT_re = re_lhsT(np.ascontiguousarray(Ws_w.T))
    vw1_re = re_lhsT(np.ascontiguousarray(V_w[:, :D].T))
    vw2_re = re_lhsT(np.ascontiguousarray(V_w[:, D:].T))
    vpt_re = re_lhsT(np.ascontiguousarray(Vp_w.T))
    wsb_re = np.ascontiguousarray(Ws_b.reshape(4, 128).T).astype(np.float32)
    vb_re = np.ascontiguousarray(V_b.reshape(4, 128).T).astype(np.float32)
    vt_re = np.ascontiguousarray(vt_w.reshape(4, 128).T).astype(BF16)
    vpb_re = Vp_b.reshape(1, V).astype(BF16)
    ones_re = np.ones((1, 128), dtype=BF16)
    ident_re = np.eye(128, dtype=np.float32).astype(BF16)

    bias2 = (b_ih + b_hh).astype(np.float32)
    x_all = emb[words]                                   # [B,T,D]
    xg_all = x_all @ W_ih.T.astype(np.float32) + bias2   # [B,T,4D]

    in_maps = []
    for k in range(NC):
        bs = slice(k * BL, (k + 1) * BL)
        xg = xg_all[bs]                                  # [2,T,2048]
        xg_re = np.ascontiguousarray(
            xg.reshape(BL, T, 16, 128).transpose(3, 2, 1, 0)
            .reshape(128, 16 * T * BL)).astype(np.float32)
        h0 = np.ascontiguousarray(
            pre_h[bs].reshape(BL, 4, 128).transpose(2, 1, 0)
            .reshape(128, 4 * BL)).astype(np.float32)
        c0 = np.ascontiguousarray(
            cell[bs].reshape(BL, 4, 128).transpose(2, 1, 0)
            .reshape(128, 4 * BL)).astype(np.float32)
        encl = enc[bs]                                   # [2,S,D]
        encT_re = np.ascontiguousarray(
            encl.reshape(BL, S, 4, 128).transpose(3, 2, 0, 1)
            .reshape(128, 4 * BL * S)).astype(BF16)
        enc_re = np.ascontiguousarray(
            encl.transpose(1, 0, 2).reshape(S, BL * D)).astype(BF16)
        in_maps.append({
            "xg": xg_re, "whh": whh_re, "h0": h0, "c0": c0,
            "encT": encT_re, "enc": enc_re, "whT": whT_re, "wsT": wsT_re,
            "vw1": vw1_re, "vw2": vw2_re, "wsb": wsb_re, "vb": vb_re,
            "vt": vt_re, "vpt": vpt_re, "vpb": vpb_re, "ones": ones_re,
            "ident": ident_re,
        })
    return in_maps


def kernel(**inputs):
    in_maps = _prep_in_maps(inputs)
    if "nc" not in _cached:
        _cached["nc"] = _build_nc()
    res = bass_utils.run_bass_kernel_spmd(_cached["nc"], in_maps,
                                          core_ids=list(range(NC)))
    outs = [res.results[k]["out"].reshape(BL, T, V) for k in range(NC)]
    return np.concatenate(outs, axis=0).astype(np.float32)


if __name__ == "__main__":
    pass
